# revision 10
# baseline (speedup 1.0000x reference)
"""HeteroMoE layer for Trainium2, 8-core SPMD.

Routing is top-1 with weight exactly 1.0, so out[b] = expert_{argmax(logits[b])}(x[b]).
Host computes routing (tiny), permutes the batch into 8 cores x 4 slots so that
each slot has a uniform compile-time "mode" (which dw-conv dilation taps / whether
the expert-2 1x1 pre-matmul is needed). All per-element selection is carried in
per-core parameter tensors (BN scale folded into weights); a zero weight makes an
unused op a no-op. Depthwise 3x3 convs run as 9 diagonal fp16 matmuls on the PE
accumulating in PSUM (together with the optional expert-2 1x1), gelu+BN-bias on
the scalar engine, the pointwise 1x1 as fp16 matmuls, bias-add on the vector
engine, all in fp32 PSUM.
"""
import numpy as np

import concourse.bacc as bacc
import concourse.tile as tile
import concourse.mybir as mybir
from concourse.bass_utils import run_bass_kernel_spmd

F32 = mybir.dt.float32
F16 = mybir.dt.float16

B, C, H, W = 32, 256, 64, 64
HW = H * W
NCORES = 8
NSLOT = B // NCORES
PAD = 2
R = W + 2 * PAD          # padded row stride (68)
TPAD = 3                 # top/bottom pad rows (extra margin for 1-D DVE runs)
RH = H + 2 * TPAD        # padded rows (70)
PADHW = R * RH
ACC0 = TPAD * R + PAD    # 1-D run start: first interior position
ACCL = (H - 1) * R + W   # 1-D run length (63*68+64 = 4348)
NBLK = C // 128          # 2 channel blocks
EPS = 1e-5

_CACHE = {}


def _offsets(dil):
    return [(dy * dil, dx * dil) for dy in (-1, 0, 1) for dx in (-1, 0, 1)]


def _slot_offsets(mode):
    if mode == "d1":
        return _offsets(1)
    if mode == "d2":
        return _offsets(2)
    if mode == "d12":
        s = _offsets(1) + [o for o in _offsets(2) if o != (0, 0)]
        return s
    return []


def build(slot_modes, tap_eng, repeat=1):
    """slot_modes: (tapmode, has_m) per slot; tap_eng: "pe"|"dve" per slot"""
    nc = bacc.Bacc("TRN2", target_bir_lowering=False, debug=False,
                   num_devices=NCORES)
    xin = nc.dram_tensor("xin", [NSLOT, C, HW], F32, kind="ExternalInput").ap()
    yout = nc.dram_tensor("yout", [NSLOT, C, HW], F32, kind="ExternalOutput").ap()
    prm = {}
    for s, (tm, has_m) in enumerate(slot_modes):
        offs = _slot_offsets(tm)
        if offs:
            if tap_eng[s] == "dve":
                prm[f"dk{s}"] = nc.dram_tensor(
                    f"dk{s}", [128, NBLK * len(offs)], F32,
                    kind="ExternalInput").ap()
            else:
                prm[f"dk{s}"] = nc.dram_tensor(
                    f"dk{s}", [128, NBLK * len(offs) * 128], F16,
                    kind="ExternalInput").ap()
        if has_m:
            prm[f"mw{s}"] = nc.dram_tensor(
                f"mw{s}", [128, NBLK * NBLK * 128], F16, kind="ExternalInput").ap()
        prm[f"pw{s}"] = nc.dram_tensor(
            f"pw{s}", [128, NBLK * NBLK * 128], F16, kind="ExternalInput").ap()
        prm[f"tb{s}"] = nc.dram_tensor(f"tb{s}", [128, NBLK], F32,
                                       kind="ExternalInput").ap()
        prm[f"qb{s}"] = nc.dram_tensor(f"qb{s}", [128, NBLK], F32,
                                       kind="ExternalInput").ap()

    with tile.TileContext(nc) as tc:
        with tc.tile_pool(name="params", bufs=1) as ppool, \
             tc.tile_pool(name="xplain", bufs=4) as xpool, \
             tc.tile_pool(name="x16", bufs=2) as cpool, \
             tc.tile_pool(name="a16", bufs=2) as apool, \
             tc.tile_pool(name="zacc", bufs=1) as zpool, \
             tc.tile_pool(name="x16o", bufs=1) as oppool, \
             tc.tile_pool(name="o32", bufs=4) as opool, \
             tc.tile_pool(name="psz", bufs=2, space="PSUM") as pszp, \
             tc.tile_pool(name="psw", bufs=4, space="PSUM") as pswp:

            # resident params
            pt = {}
            for name, ap in prm.items():
                t = ppool.tile(list(ap.shape), ap.dtype, tag=name, name=name)
                nc.sync.dma_start(t[:], ap)
                pt[name] = t

            for rep in range(repeat):
                for s, (tm, has_m) in enumerate(slot_modes):
                    offs = _slot_offsets(tm)
                    ntap = len(offs)
                    # --- load + convert to fp16 ---
                    # padded fp16 tiles (also used unpadded-interior for
                    # matmul-only slots); borders must read as zero
                    x16 = [cpool.tile([128, PADHW], F16, tag=f"x16p{_b}",
                                      name=f"x16p{_b}")
                           for _b in range(NBLK)]
                    for bk in range(NBLK):
                        if ntap:
                            nc.gpsimd.memset(x16[bk][:], 0)
                        x3 = x16[bk][:].rearrange("p (h w) -> p h w", h=RH, w=R)
                        for hf in range(2):
                            xp = xpool.tile([128, HW // 2], F32, tag="xp",
                                            name="xp")
                            nc.sync.dma_start(
                                xp[:], xin[s, bk * 128:(bk + 1) * 128,
                                           hf * (HW // 2):(hf + 1) * (HW // 2)])
                            dst = x3[:, TPAD + hf * (H // 2):
                                     TPAD + (hf + 1) * (H // 2),
                                     PAD:PAD + W]
                            src = xp[:].rearrange("p (h w) -> p h w",
                                                  h=H // 2, w=W)
                            nc.vector.tensor_copy(dst, src)

                    def rwin(bk, chunk, dy, dx):
                        ap3 = x16[bk][:].rearrange(
                            "p (h w) -> p h w", h=RH, w=R)
                        r0 = TPAD + dy + chunk * 8
                        c0 = PAD + dx
                        return ap3[:, r0:r0 + 8, c0:c0 + W]

                    # --- stage 1: z = taps + optional M@x ; gelu -> a16 ---
                    a16 = [apool.tile([128, HW], F16, tag=f"a16{_b}", name=f"a16{_b}")
                           for _b in range(NBLK)]
                    if ntap and tap_eng[s] == "dve":
                        # fp16 STT tap chain on the vector engine, 1-D runs
                        # over padded coords; odd offsets read a shift-by-one
                        # copy so the 2x packed mode always engages
                        offs1d = [dy * R + dx for (dy, dx) in offs]
                        need_odd = any(o % 2 for o in offs1d)
                        if need_odd:
                            x16o = [oppool.tile([128, PADHW], F16,
                                                tag=f"x16o{_b}", name=f"x16o{_b}")
                                    for _b in range(NBLK)]
                            for bk in range(NBLK):
                                nc.vector.tensor_copy(
                                    x16o[bk][:, 0:PADHW - 1],
                                    x16[bk][:, 1:PADHW])
                        for cb in range(NBLK):
                            acc = zpool.tile([128, PADHW], F16,
                                             tag=f"zacc{cb}", name=f"zacc{cb}")
                            for t, o in enumerate(offs1d):
                                if o % 2:
                                    src = x16o[cb][:, ACC0 + o - 1:
                                                   ACC0 + o - 1 + ACCL]
                                else:
                                    src = x16[cb][:, ACC0 + o:ACC0 + o + ACCL]
                                dkcol = pt[f"dk{s}"][:, cb * ntap + t:
                                                     cb * ntap + t + 1]
                                if t == 0:
                                    nc.vector.tensor_scalar_mul(
                                        acc[:, ACC0:ACC0 + ACCL], src, dkcol)
                                else:
                                    nc.vector.scalar_tensor_tensor(
                                        acc[:, ACC0:ACC0 + ACCL], src, dkcol,
                                        acc[:, ACC0:ACC0 + ACCL],
                                        op0=mybir.AluOpType.mult,
                                        op1=mybir.AluOpType.add)
                            zin = acc[:].rearrange(
                                "p (h w) -> p h w", h=RH, w=R)[
                                :, TPAD:TPAD + H, PAD:PAD + W]
                            nc.scalar.activation(
                                a16[cb][:].rearrange("p (h w) -> p h w",
                                                     h=H, w=W),
                                zin,
                                mybir.ActivationFunctionType.Gelu,
                                bias=pt[f"tb{s}"][:, cb:cb + 1], scale=1.0)
                        stage1_pe = False
                    else:
                        stage1_pe = True
                    for cb in (range(NBLK) if stage1_pe else []):
                        for hf4 in range(4):
                            psz = pszp.tile([128, 1024], F32, tag="psz", name="psz")
                            for q in range(2):
                                chunk = hf4 * 2 + q
                                pslice = psz[:, q * 512:(q + 1) * 512]
                                first = True
                                for t, (dy, dx) in enumerate(offs):
                                    lhsT = pt[f"dk{s}"][
                                        :, (cb * ntap + t) * 128:
                                           (cb * ntap + t + 1) * 128]
                                    nc.tensor.matmul(
                                        pslice, lhsT, rwin(cb, chunk, dy, dx),
                                        start=first,
                                        stop=(not has_m and t == ntap - 1))
                                    first = False
                                if has_m:
                                    for ib in range(NBLK):
                                        lhsT = pt[f"mw{s}"][
                                            :, (ib * NBLK + cb) * 128:
                                               (ib * NBLK + cb + 1) * 128]
                                        nc.tensor.matmul(
                                            pslice, lhsT, rwin(ib, chunk, 0, 0),
                                            start=first,
                                            stop=(ib == NBLK - 1))
                                        first = False
                            nc.scalar.activation(
                                a16[cb][:, hf4 * 1024:(hf4 + 1) * 1024],
                                psz[:],
                                mybir.ActivationFunctionType.Gelu,
                                bias=pt[f"tb{s}"][:, cb:cb + 1], scale=1.0)

                    # --- stage 2: pointwise + bias, streamed out in halves ---
                    for cb in range(NBLK):
                        for half in range(2):
                            o32 = opool.tile([128, HW // 2], F32, tag="o32",
                                             name="o32")
                            for q in range(4):
                                chunk = half * 4 + q
                                psw = pswp.tile([128, 512], F32, tag="psw",
                                                name="psw")
                                for ib in range(NBLK):
                                    lhsT = pt[f"pw{s}"][
                                        :, (ib * NBLK + cb) * 128:
                                           (ib * NBLK + cb + 1) * 128]
                                    nc.tensor.matmul(
                                        psw[:], lhsT,
                                        a16[ib][:, chunk * 512:(chunk + 1) * 512],
                                        start=(ib == 0), stop=(ib == NBLK - 1))
                                nc.scalar.activation(
                                    o32[:, q * 512:(q + 1) * 512], psw[:],
                                    mybir.ActivationFunctionType.Identity,
                                    bias=pt[f"qb{s}"][:, cb:cb + 1], scale=1.0)
                            nc.sync.dma_start(
                                yout[s, cb * 128:(cb + 1) * 128,
                                     half * (HW // 2):(half + 1) * (HW // 2)],
                                o32[:])
    nc.compile()
    return nc


def _plan(idx):
    """Assign elements to (core, slot); return slot_modes, elem[core][slot]."""
    by = [list(np.where(idx == t)[0]) for t in range(3)]
    n0, n1, n2 = map(len, by)
    groups = []  # (mode, has_m, [elems])
    for t, mode in ((0, "d1"), (1, "d2")):
        while len(by[t]) >= 8:
            groups.append([mode, False, by[t][:8]])
            by[t] = by[t][8:]
    # remainders share slots with e2 padding
    for t, mode in ((0, "d1"), (1, "d2")):
        if by[t]:
            take = min(8 - len(by[t]), len(by[2]))
            g = by[t] + by[2][:take]
            by[2] = by[2][take:]
            by[t] = []
            groups.append([mode, take > 0, g])
    while by[2]:
        groups.append([None, True, by[2][:8]])
        by[2] = by[2][8:]
    # merge if >4 groups (rare): combine two tap groups into d12
    while len(groups) > NSLOT:
        tapg = [g for g in groups if g[0] is not None]
        a, b = tapg[-2], tapg[-1]
        groups.remove(b)
        a[0] = "d12"
        a[1] = a[1] or b[1]
        a[2] += b[2]
        assert len(a[2]) <= 8
    # pad groups to exactly 8 elems (reuse element 0 as dummy -> wasted compute,
    # result discarded) and to exactly NSLOT groups
    for g in groups:
        g.append(len(g[2]))
        while len(g[2]) < 8:
            g[2].append(-1)
    while len(groups) < NSLOT:
        groups.append([None, False, [-1] * 8, 0])
    slot_modes = tuple((g[0], g[1]) for g in groups)
    elem = [[groups[s][2][c] for s in range(NSLOT)] for c in range(NCORES)]
    return slot_modes, elem


def _fold_params(kw):
    """Per expert: BN-folded weights. Returns dicts."""
    out = {}
    for i in range(3):
        g = kw[f"e{i}_g"]; b = kw[f"e{i}_b"]; m = kw[f"e{i}_m"]; v = kw[f"e{i}_v"]
        s = g / np.sqrt(v + EPS)
        t = b - m * s
        out[i] = dict(s=s.astype(np.float32), t=t.astype(np.float32),
                      pw=kw[f"e{i}_pw"].astype(np.float32),
                      pb=kw[f"e{i}_pb"].astype(np.float32))
        if i < 2:
            out[i]["k"] = (kw[f"e{i}_k"].reshape(C, 9) * s[:, None]).astype(np.float32)
        else:
            out[i]["M"] = (kw["e2_k"] * s[:, None]).astype(np.float32)
    return out


def _make_inmaps(x, idx, elem, slot_modes, tap_eng, fold):
    in_maps = []
    d1off = _slot_offsets("d1")
    for c in range(NCORES):
        im = {}
        xs = np.zeros((NSLOT, C, HW), np.float32)
        for s in range(NSLOT):
            e = elem[c][s]
            if e >= 0:
                xs[s] = x[e].reshape(C, HW)
        im["xin"] = xs
        for s, (tm, has_m) in enumerate(slot_modes):
            offs = _slot_offsets(tm)
            ntap = len(offs)
            e = elem[c][s]
            t_e = idx[e] if e >= 0 else -1
            f = fold[t_e] if t_e >= 0 else None
            if ntap:
                if tap_eng[s] == "dve":
                    dk = np.zeros((128, NBLK * ntap), np.float32)
                    if f is not None and t_e < 2:
                        myoffs = _slot_offsets("d1" if t_e == 0 else "d2")
                        for ti, o in enumerate(offs):
                            if o in myoffs:
                                ki = myoffs.index(o)
                                for bk in range(NBLK):
                                    dk[:, bk * ntap + ti] = \
                                        f["k"][bk * 128:(bk + 1) * 128, ki]
                else:
                    dk = np.zeros((128, NBLK * ntap * 128), np.float16)
                    if f is not None and t_e < 2:
                        myoffs = _slot_offsets("d1" if t_e == 0 else "d2")
                        for ti, o in enumerate(offs):
                            if o in myoffs:
                                ki = myoffs.index(o)
                                for bk in range(NBLK):
                                    col = (bk * ntap + ti) * 128
                                    dk[:, col:col + 128][np.arange(128), np.arange(128)] = \
                                        f["k"][bk * 128:(bk + 1) * 128, ki].astype(np.float16)
                im[f"dk{s}"] = dk
            if has_m:
                mw = np.zeros((128, NBLK * NBLK * 128), np.float16)
                if f is not None and t_e == 2:
                    M = f["M"]
                    for ib in range(NBLK):
                        for cb in range(NBLK):
                            blk = M[cb * 128:(cb + 1) * 128,
                                    ib * 128:(ib + 1) * 128].T
                            col = (ib * NBLK + cb) * 128
                            mw[:, col:col + 128] = blk.astype(np.float16)
                im[f"mw{s}"] = mw
            pw = np.zeros((128, NBLK * NBLK * 128), np.float16)
            tb = np.zeros((128, NBLK), np.float32)
            qb = np.zeros((128, NBLK), np.float32)
            if f is not None:
                P = f["pw"]
                for ib in range(NBLK):
                    for cb in range(NBLK):
                        blk = P[cb * 128:(cb + 1) * 128,
                                ib * 128:(ib + 1) * 128].T
                        col = (ib * NBLK + cb) * 128
                        pw[:, col:col + 128] = blk.astype(np.float16)
                tb[:] = f["t"].reshape(NBLK, 128).T
                qb[:] = f["pb"].reshape(NBLK, 128).T
            im[f"pw{s}"] = pw
            im[f"tb{s}"] = tb
            im[f"qb{s}"] = qb
        in_maps.append(im)
    return in_maps


def kernel(**inputs):
    x = np.ascontiguousarray(inputs["x"], np.float32)
    rw = np.asarray(inputs["rw"], np.float32)
    rb = np.asarray(inputs["rb"], np.float32)
    pooled = x.mean(axis=(2, 3), dtype=np.float32)
    logits = pooled @ rw.T + rb
    idx = logits.argmax(-1)

    slot_modes, elem = _plan(idx)
    tap_eng = tuple("pe" for _ in slot_modes)
    fold = _fold_params(inputs)
    in_maps = _make_inmaps(x, idx, elem, slot_modes, tap_eng, fold)

    key = (slot_modes, tap_eng)
    if key not in _CACHE:
        _CACHE[key] = build(slot_modes, tap_eng)
    nc = _CACHE[key]
    res = run_bass_kernel_spmd(nc, in_maps, core_ids=list(range(NCORES)),
                               trace=False)
    out = np.zeros((B, C, H, W), np.float32)
    for c in range(NCORES):
        yo = res.results[c]["yout"]
        for s in range(NSLOT):
            e = elem[c][s]
            if e >= 0:
                out[e] = yo[s].reshape(C, H, W)
    return out
